# revision 53
# baseline (speedup 1.0000x reference)
"""MatchBRNN Trainium2 kernel: 2-layer action-conditioned-attention + bidirectional
SRU, data-parallel over batch on 8 NeuronCores (B=16 -> 2 batches/core).

v3: transfer- and overlap-optimized.
- All inputs packed into ONE bf16 blob per core (~659KB vs 5.5MB f32 in v1,
  8x less host->device traffic); outputs return in bf16.
- The replicated SRU weight matrix is sharded 8-way and AllGather-ed on
  device (2 gathers, one per layer, overlapped with attention compute).
- memr (the l-major copy of x used by the pools matmul) is derived on-device
  by PE transpose-mode matmuls instead of being shipped.
- The attention projection matmuls use PE column-tiling (out base_partition
  = b*64) instead of zero-padded block-diagonal weights; all matmul inputs
  are bf16 (FWL weight loads).
- One global instruction schedule across both layers: chunk-1 tanh blocks
  cover chunk-0's softmax/SRU tail, and layer-1's ytT + first tanh blocks
  are woven into layer-0's tail so the ACT engine (the bottleneck) never
  drains at chunk/layer boundaries. Simulated exec: ~159us vs ~188us for v1.

Layout C: on-chip column index for (position q, batch b) is
    C(q, b) = (q // 128) * 256 + b * 128 + (q % 128)
i.e. 128-position chunks, batch-major inside a chunk. The kernel streams the
softmax/pools/SRU tail per 128-position chunk behind the tanh/score pipeline
of the next chunk.

Per-core pipeline, per layer:
  xtT[(b,k), l] = (x_b @ w1a_b)^T          (col-tiled over b; layer-invariant)
  ytT[(b,k), s] = (out_b @ w2a_b)^T + (b1a + b2a)
  per chunk ck (128 s values):
    t = tanh(xtT + ytT[:, s]) -> bf16      (DVE per-s add + big ACT tanh)
    scoresT[l, C(s,b)] = t^T blockdiag(va) (per-s PE matmuls, FD=2)
    e = exp(scoresT); Z = ones^T e; rz = 1/Z; pools = (mem^T e) * rz
    U_j = W_j^T [out; pools]               (bf16 matmuls, 4 gate slices)
    f,r via tanh; c = scan(f, (1-f)u0); h = r*tanh(c) + (1-r)*hw
"""
import numpy as np
import concourse.bass as bass
import concourse.mybir as mybir
import concourse.tile as tile
from concourse.bass_utils import run_bass_kernel_spmd

AF = mybir.ActivationFunctionType
OP = mybir.AluOpType
F32 = mybir.dt.float32
BF16 = mybir.dt.bfloat16
BF16_NP = mybir.dt.np(BF16)

B, S, D = 16, 256, 256
H, NL, A, K = 128, 2, 8, 64
NCORES = 8
B2 = B // NCORES
USE_ALLGATHER = True
# AllGather groups as (shard-col offset, width): one 1MB gather per layer's
# weights, so layer-0 weights land while layer-0 attention is still running.
GROUPS = ((0, 512), (512, 512))

# blob column map (all bf16); memr is derived on-device by PE-transposes of
# the memT blocks, so it is not shipped.
C_MEMT = 0          # [128, 1024]
C_WSH = 1024        # [128, 1024] wsru shard (allgather) or [128, 8192] full
C_W1 = None         # set below
C_W2 = None
C_VA = None
C_YB = None
C_BS = None
C_MK = None
C_END = None


def _set_cols():
    global C_W1, C_W2, C_VA, C_YB, C_BS, C_MK, C_END
    base = C_WSH + (1024 if USE_ALLGATHER else 8192)
    C_W1 = base
    C_W2 = base + 256
    C_VA = base + 512
    C_YB = base + 514
    C_BS = base + 515
    C_MK = base + 523
    C_END = base + 527


_set_cols()


def _split_excess_waits(nc, max_waits=1):
    """walrus in this toolchain rejects >1 sem-wait per instruction; hoist
    extras onto same-engine NoOps inserted just before the instruction."""
    n = 0
    for f in nc.m.functions:
        for bb in f.blocks:
            out = []
            for inst in bb.instructions:
                si = inst.sync_info
                waits = list(si.on_wait) if si is not None and si.on_wait else []
                if len(waits) > max_waits:
                    keep, extra = waits[-max_waits:], waits[:-max_waits]
                    for w in extra:
                        n += 1
                        out.append(mybir.InstNoOp(
                            name=f"{inst.name}_ws{n}", engine=inst.engine,
                            ins=[], outs=[],
                            sync_info=mybir.SyncInfo(on_wait=[w], on_update=[])))
                    inst.sync_info = mybir.SyncInfo(
                        on_wait=keep, on_update=list(si.on_update or []))
                out.append(inst)
            bb.instructions = out
    return n


def _build(apply_mask: bool):
    nc = bass.Bass("TRN2", num_devices=NCORES)
    dram = nc.dram_tensor
    blob_d = dram("blob", [128, C_END], BF16, kind="ExternalInput")
    outT_d = dram("outT", [2, 128, 512], BF16, kind="ExternalOutput")

    with tile.TileContext(nc) as tc:
        with (
            nc.allow_low_precision(reason="bf16 staging is intentional"),
            tc.tile_pool(name="const", bufs=1) as cp,
            tc.tile_pool(name="work", bufs=1) as wp,
            tc.tile_pool(name="blk", bufs=3) as bp,
            tc.tile_pool(name="sru", bufs=2) as sp,
            tc.tile_pool(name="ps", bufs=1, space="PSUM") as ps,
            tc.tile_pool(name="dramp", bufs=1, space="DRAM") as dpool,
        ):
            # ACT table preload: tiny tanh right at t=0, concurrent with DMAs
            warm = cp.tile([128, 1], F32, tag="warm")
            nc.vector.memset(warm[:], 0.0)
            nc.scalar.activation(warm[:], warm[:], AF.Tanh)

            memT = cp.tile([128, 1024], BF16, tag="memT")
            memr = cp.tile([128, 1024], F32, tag="memr")
            ident = cp.tile([128, 128], F32, tag="ident")
            smalls = cp.tile([128, C_END - C_W1], BF16, tag="smalls")
            yb = cp.tile([128, 1], F32, tag="yb")
            bsru = cp.tile([128, 8], F32, tag="bsru")
            onc = cp.tile([128, 1], F32, tag="onc")
            onr = cp.tile([1, 128], F32, tag="onr")
            wsru = cp.tile([128, 8192], BF16, tag="wsru")
            # offsets inside smalls
            W1O, W2O = 0, 256
            VAO, YBO, BSO, MKO = 512, 514, 515, 523

            # priority order: smalls first (feeds the stationary operands),
            # then memT
            nc.sync.dma_start(smalls[:], blob_d[:, C_W1:C_END])
            for q in range(2):
                nc.sync.dma_start(
                    memT[:, q * 512:(q + 1) * 512],
                    blob_d[:, C_MEMT + q * 512:C_MEMT + (q + 1) * 512])
            nc.vector.memset(ident[:], 1.0)
            nc.gpsimd.affine_select(ident[:], ident[:], [[1, 128]],
                                    OP.is_equal, 0.0, base=0,
                                    channel_multiplier=-1)
            nc.vector.memset(onc[:], 1.0)
            nc.vector.memset(onr[:], 1.0)
            nc.vector.tensor_copy(yb[:], smalls[:, YBO:YBO + 1])
            nc.vector.tensor_copy(bsru[:], smalls[:, BSO:BSO + 8])
            if apply_mask:
                mk = cp.tile([128, 4], F32, tag="mk")
                nc.vector.tensor_copy(mk[:], smalls[:, MKO:MKO + 4])

            # SRU weights: 8-way sharded + four on-device AllGathers (one per
            # (layer, direction) weight group, arriving just in time each)
            if USE_ALLGATHER:
                # gathers sized to land just in time (see GROUPS)
                gaths = []
                for p, (goff, gw) in enumerate(GROUPS):
                    ws_in = dpool.tile([128, gw], BF16, name=f"ws_in{p}")
                    ws_gath = dpool.tile([1024, gw], BF16,
                                         name=f"ws_gath{p}",
                                         addr_space="Shared")
                    gaths.append(ws_gath)
                    nc.gpsimd.dma_start(
                        ws_in[:],
                        blob_d[:, C_WSH + goff:C_WSH + goff + gw])
                    nc.gpsimd.collective_compute(
                        "AllGather", OP.bypass,
                        replica_groups=[list(range(NCORES))],
                        ins=[ws_in[:].rearrange("p f -> (p f)")],
                        outs=[ws_gath[:].rearrange("p f -> (p f)")])
                # gather->SBUF loads after ALL collective dispatches, so the
                # later collectives' dispatch isn't blocked behind loads that
                # wait on earlier collectives (Pool queue is in-order)
                for p, (goff, gw) in enumerate(GROUPS):
                    for r in range(NCORES):
                        nc.gpsimd.dma_start(
                            wsru[:, goff * 8 + r * gw:goff * 8 + (r + 1) * gw],
                            gaths[p][r * 128:(r + 1) * 128, :])
            else:
                for q in range(4):
                    nc.sync.dma_start(
                        wsru[:, q * 2048:(q + 1) * 2048],
                        blob_d[:, C_WSH + q * 2048:C_WSH + (q + 1) * 2048])

            h0 = [wp.tile([128, 512], BF16, tag=f"h0{d}", name=f"h0{d}")
                  for d in range(2)]
            h1 = [wp.tile([128, 512], BF16, tag=f"h1{d}", name=f"h1{d}")
                  for d in range(2)]

            # PSUM: 8 banks, all as (128, 512) f32 tiles
            u_ps = {}
            for jj in range(4):
                u_ps[jj] = ps.tile([128, 512], F32, tag=f"u{jj}", name=f"ups{jj}")
            sc_ps = [ps.tile([128, 512], F32, tag=f"sc{h}", name=f"scps{h}")
                     for h in range(2)]
            pn_ps = [ps.tile([128, 512], F32, tag=f"pn{dh}", name=f"pnps{dh}")
                     for dh in range(2)]

            # memr[lp, lh*512+b*256+dh*128+dc] = x[b, lh*128+lp, dh*128+dc]
            #   = PE-transpose of memT block [dp, q] at cols dh*512+lh*256+b*128
            # (emitted in the schedule after the first tanh blocks, so its DVE
            # copies don't delay the first tp adds; needed by pools at ~35us)
            def emit_memr():
                ti = 0
                for lh in range(2):
                    for b in range(2):
                        for dh in range(2):
                            tps = u_ps[ti % 4]
                            src = dh * 512 + lh * 256 + b * 128
                            dst = lh * 512 + b * 256 + dh * 128
                            t32 = bp.tile([128, 128], F32, tag="tsp",
                                          name=f"tsp{ti}")
                            nc.vector.tensor_copy(t32[:],
                                                  memT[:, src:src + 128])
                            nc.tensor.transpose(tps[:, 0:128], t32[:],
                                                ident[:])
                            nc.vector.tensor_copy(memr[:, dst:dst + 128],
                                                  tps[:, 0:128])
                            ti += 1

            # xtT (layer-invariant): contract d per b-half via col-tiling.
            # Staged per 128-l chunk through sc_ps[0] before any scores land.
            xt16 = wp.tile([128, 256], BF16, tag="xt16")
            for ck in range(2):
                co = ck * 256
                for b in range(2):
                    for ci in range(2):
                        nc.tensor.matmul(
                            sc_ps[0][b * 64:(b + 1) * 64, co:co + 128],
                            smalls[:, W1O + (b * 2 + ci) * 64:
                                   W1O + (b * 2 + ci + 1) * 64],
                            memT[:, ci * 512 + co + b * 128:
                                 ci * 512 + co + (b + 1) * 128],
                            start=(ci == 0), stop=(ci == 1))
                nc.vector.tensor_copy(xt16[:, ck * 128:(ck + 1) * 128],
                                      sc_ps[0][:, co:co + 128])

            # per-layer tiles (distinct buffers so layers can overlap)
            yts = [wp.tile([128, 256], F32, tag=f"yt{li}", name=f"yt{li}")
                   for li in range(NL)]
            eTs = [wp.tile([128, 1024], F32, tag=f"eT{li}", name=f"eT{li}")
                   for li in range(NL)]
            rzs = [wp.tile([1, 512], F32, tag=f"rz{li}", name=f"rz{li}")
                   for li in range(NL)]
            rzbs = [wp.tile([128, 512], F32, tag=f"rzb{li}", name=f"rzb{li}")
                    for li in range(NL)]
            poolsTs = [[wp.tile([128, 512], BF16, tag=f"poolsT{li}_{dh}",
                                name=f"poolsT{li}_{dh}") for dh in range(2)]
                       for li in range(NL)]
            gates = {}

            def emit_ytT(li, ck):
                # staged in sc_ps[1][:, ck*256 : ck*256+128]; freed by the yt
                # copy before this layer's chunk-ck score MMs land
                co = ck * 256
                for b in range(2):
                    for ci in range(2):
                        if li == 0:
                            rhs = memT[:, ci * 512 + co + b * 128:
                                       ci * 512 + co + (b + 1) * 128]
                        else:
                            rhs = h0[ci][:, co + b * 128: co + (b + 1) * 128]
                        nc.tensor.matmul(
                            sc_ps[1][b * 64:(b + 1) * 64, co:co + 128],
                            smalls[:, W2O + (b * 2 + ci) * 64:
                                   W2O + (b * 2 + ci + 1) * 64],
                            rhs, start=(ci == 0), stop=(ci == 1))
                nc.vector.tensor_scalar(
                    yts[li][:, ck * 128:(ck + 1) * 128],
                    sc_ps[1][:, co:co + 128], yb[:], None, OP.add)

            def emit_block(li, ck, blk):
                co = ck * 256
                tp = bp.tile([128, 4096], BF16, tag="tpre",
                             name=f"tp{li}_{ck}_{blk}")
                tb = bp.tile([128, 4096], BF16, tag="tblk",
                             name=f"tb{li}_{ck}_{blk}")
                for j in range(16):
                    s = ck * 128 + blk * 16 + j
                    nc.vector.tensor_scalar(
                        tp[:, j * 256:(j + 1) * 256], xt16[:],
                        yts[li][:, s:s + 1], None, OP.add)
                nc.scalar.activation(tb[:], tp[:], AF.Tanh)
                for j in range(16):
                    q = blk * 16 + j
                    for h in range(2):
                        # out cols {co+q, co+128+q}: C-layout b-split
                        nc.tensor.matmul(
                            sc_ps[h][:, co + q: co + q + 129: 128],
                            tb[:, j * 256 + h * 128: j * 256 + (h + 1) * 128],
                            smalls[:, VAO:VAO + 2],
                            start=True, stop=True)

            def emit_pools(li, ck):
                co = ck * 256
                eT, rz, rzb = eTs[li], rzs[li], rzbs[li]
                for h in range(2):
                    nc.scalar.activation(eT[:, h * 512 + co: h * 512 + co + 256],
                                         sc_ps[h][:, co:co + 256], AF.Exp)
                if apply_mask:
                    for h in range(2):
                        for b in range(2):
                            sl = eT[:, h * 512 + co + b * 128:
                                    h * 512 + co + (b + 1) * 128]
                            nc.vector.tensor_scalar(
                                sl, sl, mk[:, h * 2 + b: h * 2 + b + 1],
                                None, OP.mult)
                for h in range(2):
                    nc.tensor.matmul(pn_ps[0][0:1, co:co + 256], onc[:],
                                     eT[:, h * 512 + co: h * 512 + co + 256],
                                     start=(h == 0), stop=(h == 1))
                nc.vector.reciprocal(rz[0:1, co:co + 256],
                                     pn_ps[0][0:1, co:co + 256])
                for b in range(2):
                    nc.tensor.matmul(
                        pn_ps[1][:, co + b * 128: co + (b + 1) * 128], onr[:],
                        rz[0:1, co + b * 128: co + (b + 1) * 128],
                        start=True, stop=True)
                nc.vector.tensor_copy(rzb[:, co:co + 256],
                                      pn_ps[1][:, co:co + 256])
                for dh in range(2):
                    for b in range(2):
                        for lh in range(2):
                            nc.tensor.matmul(
                                pn_ps[dh][:, co + b * 128: co + (b + 1) * 128],
                                memr[:, lh * 512 + b * 256 + dh * 128:
                                     lh * 512 + b * 256 + (dh + 1) * 128],
                                eT[:, lh * 512 + co + b * 128:
                                   lh * 512 + co + (b + 1) * 128],
                                start=(lh == 0), stop=(lh == 1))
                    nc.vector.scalar_tensor_tensor(
                        poolsTs[li][dh][:, co:co + 256],
                        pn_ps[dh][:, co:co + 256],
                        1.0, rzb[:, co:co + 256], OP.mult, OP.mult)

            def emit_sru(li, ck, dr):
                co = ck * 256
                for c in range(4):
                    if c < 2:
                        rhs = (memT[:, c * 512 + co: c * 512 + co + 256]
                               if li == 0 else h0[c][:, co:co + 256])
                    else:
                        rhs = poolsTs[li][c - 2][:, co:co + 256]
                    for jj in range(4):
                        w_off = (((li * 2 + dr) * 16) + c * 4 + jj) * 128
                        nc.tensor.matmul(
                            u_ps[jj][:, co:co + 256],
                            wsru[:, w_off:w_off + 128], rhs,
                            start=(c == 0), stop=(c == 3))
                bcol = (li * 2 + dr) * 2
                if (li, dr) not in gates:
                    gt = {}
                    for nm in ("tf", "f", "g", "bin", "c", "tc2", "tr",
                               "dd", "rd2"):
                        gt[nm] = sp.tile([128, 512], F32, tag=nm,
                                         name=f"{nm}_{li}_{dr}")
                    gates[(li, dr)] = gt
                gt = gates[(li, dr)]
                tf_, f_, g_, bin_, c_, tc2, tr_, dd_, rd2_ = (
                    gt["tf"], gt["f"], gt["g"], gt["bin"], gt["c"],
                    gt["tc2"], gt["tr"], gt["dd"], gt["rd2"])
                nc.scalar.activation(tf_[:, co:co + 256],
                                     u_ps[1][:, co:co + 256], AF.Tanh,
                                     bias=bsru[:, bcol:bcol + 1], scale=0.5)
                nc.vector.tensor_scalar(f_[:, co:co + 256],
                                        tf_[:, co:co + 256], 0.5, 0.5,
                                        OP.mult, OP.add)
                nc.vector.tensor_scalar(g_[:, co:co + 256],
                                        tf_[:, co:co + 256], -0.5, 0.5,
                                        OP.mult, OP.add)
                nc.vector.tensor_tensor(bin_[:, co:co + 256],
                                        g_[:, co:co + 256],
                                        u_ps[0][:, co:co + 256], OP.mult)
                for b in range(2):
                    lo = co + b * 128
                    init = (0.0 if ck == 0
                            else c_[:, lo - 129: lo - 128])
                    nc.vector.tensor_tensor_scan(
                        c_[:, lo:lo + 128], f_[:, lo:lo + 128],
                        bin_[:, lo:lo + 128], init, OP.mult, OP.add)
                nc.scalar.activation(tc2[:, co:co + 256],
                                     c_[:, co:co + 256], AF.Tanh)
                nc.scalar.activation(tr_[:, co:co + 256],
                                     u_ps[2][:, co:co + 256], AF.Tanh,
                                     bias=bsru[:, bcol + 1:bcol + 2],
                                     scale=0.5)
                nc.vector.tensor_tensor(dd_[:, co:co + 256],
                                        tc2[:, co:co + 256],
                                        u_ps[3][:, co:co + 256],
                                        OP.subtract)
                nc.vector.scalar_tensor_tensor(
                    rd2_[:, co:co + 256], tr_[:, co:co + 256], 1.0,
                    dd_[:, co:co + 256], OP.add, OP.mult)
                h_t = h0[dr] if li == 0 else h1[dr]
                nc.vector.scalar_tensor_tensor(
                    h_t[:, co:co + 256], rd2_[:, co:co + 256], 0.5,
                    u_ps[3][:, co:co + 256], OP.mult, OP.add)

            def emit_out(ck):
                co = ck * 256
                for dh in range(2):
                    nc.sync.dma_start(outT_d[dh, :, co:co + 256],
                                      h1[dh][:, co:co + 256])

            # global schedule across both layers: chunk-1 blocks cover the
            # chunk-0 tails; layer-1 ytT + first blocks are woven into layer
            # 0's tail so ACT never drains at the layer boundary.
            emit_ytT(0, 0)
            emit_ytT(0, 1)
            for blk in range(8):
                emit_block(0, 0, blk)
                if blk == 1:
                    emit_memr()
            emit_block(0, 1, 0)
            emit_block(0, 1, 1)
            emit_pools(0, 0)
            for blk in range(2, 5):
                emit_block(0, 1, blk)
            emit_sru(0, 0, 0)
            emit_sru(0, 0, 1)
            for blk in range(5, 8):
                emit_block(0, 1, blk)
            emit_ytT(1, 0)
            emit_pools(0, 1)
            emit_sru(0, 1, 0)
            emit_sru(0, 1, 1)
            emit_block(1, 0, 0)
            emit_block(1, 0, 1)
            emit_ytT(1, 1)
            for blk in range(2, 8):
                emit_block(1, 0, blk)
            emit_block(1, 1, 0)
            emit_block(1, 1, 1)
            emit_pools(1, 0)
            emit_sru(1, 0, 0)
            emit_sru(1, 0, 1)
            emit_out(0)
            for blk in range(2, 8):
                emit_block(1, 1, blk)
            emit_pools(1, 1)
            emit_sru(1, 1, 0)
            emit_sru(1, 1, 1)
            emit_out(1)

    _split_excess_waits(nc)
    return nc


_CACHE = {}


def _get_nc(apply_mask: bool):
    if apply_mask not in _CACHE:
        _CACHE[apply_mask] = _build(apply_mask)
    return _CACHE[apply_mask]


def _c_layout(arr_pos_b):
    """(pos, b, ...) -> columns in layout C: [ck*256 + b*128 + q]."""
    P2, Bb = arr_pos_b.shape[0], arr_pos_b.shape[1]
    rest = arr_pos_b.shape[2:]
    a = arr_pos_b.reshape(2, 128, Bb, *rest)       # (ck, q, b, ...)
    a = np.moveaxis(a, 2, 1)                       # (ck, b, q, ...)
    return a.reshape(512, *rest)


def make_in_maps(x, x_mask, actions, w1, b1, w2, b2, v,
                 sru_w_f, sru_b_f, sru_w_b, sru_b_b):
    x = np.asarray(x, np.float32)
    x_mask = np.asarray(x_mask)
    actions = np.asarray(actions).astype(np.int64)
    w1 = np.asarray(w1, np.float32); b1 = np.asarray(b1, np.float32)
    w2 = np.asarray(w2, np.float32); b2 = np.asarray(b2, np.float32)
    v = np.asarray(v, np.float32)
    sru_w = [np.asarray(sru_w_f, np.float32), np.asarray(sru_w_b, np.float32)]
    sru_b = [np.asarray(sru_b_f, np.float32), np.asarray(sru_b_b, np.float32)]

    apply_mask = bool(x_mask.any())

    # full wsru weight layout [128, 8192]: 64 blocks of 128 cols keyed
    # (li, dr, c, jj); sharded per core as cols [r*1024:(r+1)*1024]
    wsru_full = np.empty((128, 8192), np.float32)
    for li in range(NL):
        for dr in range(2):
            blk = sru_w[dr][li].reshape(4, 128, 4, 128)  # [c, dp, jj, m]
            for c in range(4):
                for jj in range(4):
                    off = ((li * 2 + dr) * 16 + c * 4 + jj) * 128
                    wsru_full[:, off:off + 128] = blk[c, :, jj, :]
    wsru16 = wsru_full.astype(BF16_NP)
    bsru = np.empty((128, 8), np.float32)
    for li in range(NL):
        for dr in range(2):
            bb = sru_b[dr][li]
            bsru[:, (li * 2 + dr) * 2 + 0] = 0.5 * bb[0:128]
            bsru[:, (li * 2 + dr) * 2 + 1] = 0.5 * bb[128:256]
    bsru16 = bsru.astype(BF16_NP)

    in_maps = []
    for core in range(NCORES):
        gb = [B2 * core + b for b in range(B2)]
        xs = x[gb]  # (2, S, D)
        a = [int(actions[g]) for g in gb]
        blob = np.zeros((128, C_END), np.float32)
        # memT[dp, dh*512 + C(l, b)] = x[b, l, dh*128+dp]
        arr = xs.transpose(1, 0, 2)                # (l, b, d)
        colsC = _c_layout(arr)                     # (512C, d)
        for dh in range(2):
            blob[:, C_MEMT + dh * 512:C_MEMT + (dh + 1) * 512] = \
                colsC[:, dh * 128:(dh + 1) * 128].T
        # w dense: [128 dp(ci-half), (b,ci) block * 64 + k]
        for b in range(2):
            for ci in range(2):
                cc = b * 2 + ci
                blob[:, C_W1 + cc * 64:C_W1 + (cc + 1) * 64] = \
                    w1[a[b], ci * 128:(ci + 1) * 128, :]
                blob[:, C_W2 + cc * 64:C_W2 + (cc + 1) * 64] = \
                    w2[a[b], ci * 128:(ci + 1) * 128, :]
        for b in range(2):
            blob[b * 64:(b + 1) * 64, C_VA + b] = v[a[b]]
            blob[b * 64:(b + 1) * 64, C_YB] = b1[a[b]] + b2[a[b]]
        blob[:, C_BS:C_BS + 8] = bsru
        if apply_mask:
            for lh in range(2):
                for b in range(2):
                    blob[:, C_MK + lh * 2 + b] = np.where(
                        x_mask[gb[b], lh * 128:(lh + 1) * 128], 0.0, 1.0)
        blob16 = blob.astype(BF16_NP)
        if USE_ALLGATHER:
            # shard-col goff..goff+gw maps to wsru block cols goff*8..
            for goff, gw in GROUPS:
                blob16[:, C_WSH + goff:C_WSH + goff + gw] = \
                    wsru16[:, goff * 8 + core * gw:goff * 8 + (core + 1) * gw]
        else:
            blob16[:, C_WSH:C_WSH + 8192] = wsru16
        blob16[:, C_BS:C_BS + 8] = bsru16
        in_maps.append({"blob": blob16})
    return in_maps, apply_mask


def assemble_output(results):
    y = np.empty((B, S, D), np.float32)
    for core in range(NCORES):
        outT = np.asarray(results[core]["outT"]).astype(np.float32)
        oc = outT.reshape(2, 128, 2, 2, 128)       # [dh, dp, ck, b, q]
        for b in range(B2):
            # y[b, s, dh*128+dp]; s = ck*128+q
            yb = oc[:, :, :, b, :]                 # (dh, dp, ck, q)
            yb = yb.transpose(2, 3, 0, 1).reshape(S, D)
            y[B2 * core + b] = yb
    return y


def kernel(**inputs) -> np.ndarray:
    in_maps, apply_mask = make_in_maps(**inputs)
    nc = _get_nc(apply_mask)
    res = run_bass_kernel_spmd(nc, in_maps, list(range(NCORES)))
    return assemble_output(res.results)


# revision 69
# speedup vs baseline: 1.0260x; 1.0260x over previous
"""MatchBRNN Trainium2 kernel: 2-layer action-conditioned-attention + bidirectional
SRU, data-parallel over batch on 8 NeuronCores (B=16 -> 2 batches/core).

v3: transfer- and overlap-optimized.
- All inputs packed into ONE bf16 blob per core (~659KB vs 5.5MB f32 in v1,
  8x less host->device traffic); outputs return in bf16.
- The replicated SRU weight matrix is sharded 8-way and AllGather-ed on
  device (2 gathers, one per layer, overlapped with attention compute).
- memr (the l-major copy of x used by the pools matmul) is derived on-device
  by PE transpose-mode matmuls instead of being shipped.
- The attention projection matmuls use PE column-tiling (out base_partition
  = b*64) instead of zero-padded block-diagonal weights; all matmul inputs
  are bf16 (FWL weight loads).
- One global instruction schedule across both layers: chunk-1 tanh blocks
  cover chunk-0's softmax/SRU tail, and layer-1's ytT + first tanh blocks
  are woven into layer-0's tail so the ACT engine (the bottleneck) never
  drains at chunk/layer boundaries. Simulated exec: ~159us vs ~188us for v1.

Layout C: on-chip column index for (position q, batch b) is
    C(q, b) = (q // 128) * 256 + b * 128 + (q % 128)
i.e. 128-position chunks, batch-major inside a chunk. The kernel streams the
softmax/pools/SRU tail per 128-position chunk behind the tanh/score pipeline
of the next chunk.

Per-core pipeline, per layer:
  xtT[(b,k), l] = (x_b @ w1a_b)^T          (col-tiled over b; layer-invariant)
  ytT[(b,k), s] = (out_b @ w2a_b)^T + (b1a + b2a)
  per chunk ck (128 s values):
    t = tanh(xtT + ytT[:, s]) -> bf16      (DVE per-s add + big ACT tanh)
    scoresT[l, C(s,b)] = t^T blockdiag(va) (per-s PE matmuls, FD=2)
    e = exp(scoresT); Z = ones^T e; rz = 1/Z; pools = (mem^T e) * rz
    U_j = W_j^T [out; pools]               (bf16 matmuls, 4 gate slices)
    f,r via tanh; c = scan(f, (1-f)u0); h = r*tanh(c) + (1-r)*hw
"""
import numpy as np
import concourse.bass as bass
import concourse.mybir as mybir
import concourse.tile as tile
from concourse.bass_utils import run_bass_kernel_spmd

AF = mybir.ActivationFunctionType
OP = mybir.AluOpType
F32 = mybir.dt.float32
BF16 = mybir.dt.bfloat16
BF16_NP = mybir.dt.np(BF16)

B, S, D = 16, 256, 256
H, NL, A, K = 128, 2, 8, 64
NCORES = 8
B2 = B // NCORES
USE_ALLGATHER = True
# AllGather groups as (shard-col offset, width): one 1MB gather per layer's
# weights, so layer-0 weights land while layer-0 attention is still running.
GROUPS = ((0, 512), (512, 512))
NJ = 16       # s-values per tanh block (tanh FD = 256*NJ)
TPBUFS = 4    # tp/tb buffering (4 lets the scheduler order all tanh-block
              # adds ahead of the collective-gated SRU gate ops)

# blob column map (all bf16); memr is derived on-device by PE-transposes of
# the memT blocks, so it is not shipped.
C_MEMT = 0          # [128, 1024]
C_WSH = 1024        # [128, 1024] wsru shard (allgather) or [128, 8192] full
C_W1 = None         # set below
C_W2 = None
C_VA = None
C_YB = None
C_BS = None
C_MK = None
C_END = None


def _set_cols():
    global C_W1, C_W2, C_VA, C_YB, C_BS, C_MK, C_END
    base = C_WSH + (1024 if USE_ALLGATHER else 8192)
    C_W1 = base
    C_W2 = base + 256
    C_VA = base + 512
    C_YB = base + 514
    C_BS = base + 515
    C_MK = base + 523
    C_END = base + 527


_set_cols()


def _split_excess_waits(nc, max_waits=1):
    """walrus in this toolchain rejects >1 sem-wait per instruction; hoist
    extras onto same-engine NoOps inserted just before the instruction."""
    n = 0
    for f in nc.m.functions:
        for bb in f.blocks:
            out = []
            for inst in bb.instructions:
                si = inst.sync_info
                waits = list(si.on_wait) if si is not None and si.on_wait else []
                if len(waits) > max_waits:
                    keep, extra = waits[-max_waits:], waits[:-max_waits]
                    for w in extra:
                        n += 1
                        out.append(mybir.InstNoOp(
                            name=f"{inst.name}_ws{n}", engine=inst.engine,
                            ins=[], outs=[],
                            sync_info=mybir.SyncInfo(on_wait=[w], on_update=[])))
                    inst.sync_info = mybir.SyncInfo(
                        on_wait=keep, on_update=list(si.on_update or []))
                out.append(inst)
            bb.instructions = out
    return n


def _build(apply_mask: bool):
    nc = bass.Bass("TRN2", num_devices=NCORES)
    dram = nc.dram_tensor
    blob_d = dram("blob", [128, C_END], BF16, kind="ExternalInput")
    outT_d = dram("outT", [2, 128, 512], BF16, kind="ExternalOutput")

    with tile.TileContext(nc) as tc:
        with (
            nc.allow_low_precision(reason="bf16 staging is intentional"),
            tc.tile_pool(name="const", bufs=1) as cp,
            tc.tile_pool(name="work", bufs=1) as wp,
            tc.tile_pool(name="blk", bufs=3) as bp,
            tc.tile_pool(name="sru", bufs=2) as sp,
            tc.tile_pool(name="ps", bufs=1, space="PSUM") as ps,
            tc.tile_pool(name="dramp", bufs=1, space="DRAM") as dpool,
        ):
            # ACT table preload: tiny tanh right at t=0, concurrent with DMAs
            warm = cp.tile([128, 1], F32, tag="warm")
            nc.vector.memset(warm[:], 0.0)
            nc.scalar.activation(warm[:], warm[:], AF.Tanh)

            memT = cp.tile([128, 1024], BF16, tag="memT")
            memr = cp.tile([128, 1024], F32, tag="memr")
            ident = cp.tile([128, 128], F32, tag="ident")
            smalls = cp.tile([128, C_END - C_W1], BF16, tag="smalls")
            yb = cp.tile([128, 1], F32, tag="yb")
            bsru = cp.tile([128, 8], F32, tag="bsru")
            onc = cp.tile([128, 1], F32, tag="onc")
            onr = cp.tile([1, 128], F32, tag="onr")
            wsru = cp.tile([128, 8192], BF16, tag="wsru")
            # offsets inside smalls
            W1O, W2O = 0, 256
            VAO, YBO, BSO, MKO = 512, 514, 515, 523

            # priority order: smalls first (feeds the stationary operands),
            # then memT
            nc.sync.dma_start(smalls[:], blob_d[:, C_W1:C_END])
            for q in range(2):
                nc.sync.dma_start(
                    memT[:, q * 512:(q + 1) * 512],
                    blob_d[:, C_MEMT + q * 512:C_MEMT + (q + 1) * 512])
            nc.vector.memset(ident[:], 1.0)
            nc.gpsimd.affine_select(ident[:], ident[:], [[1, 128]],
                                    OP.is_equal, 0.0, base=0,
                                    channel_multiplier=-1)
            nc.vector.memset(onc[:], 1.0)
            nc.vector.memset(onr[:], 1.0)
            nc.vector.tensor_copy(yb[:], smalls[:, YBO:YBO + 1])
            nc.vector.tensor_copy(bsru[:], smalls[:, BSO:BSO + 8])
            if apply_mask:
                mk = cp.tile([128, 4], F32, tag="mk")
                nc.vector.tensor_copy(mk[:], smalls[:, MKO:MKO + 4])

            # SRU weights: 8-way sharded + four on-device AllGathers (one per
            # (layer, direction) weight group, arriving just in time each)
            if USE_ALLGATHER:
                # gathers sized to land just in time (see GROUPS)
                gaths = []
                for p, (goff, gw) in enumerate(GROUPS):
                    ws_in = dpool.tile([128, gw], BF16, name=f"ws_in{p}")
                    ws_gath = dpool.tile([1024, gw], BF16,
                                         name=f"ws_gath{p}",
                                         addr_space="Shared")
                    gaths.append(ws_gath)
                    nc.gpsimd.dma_start(
                        ws_in[:],
                        blob_d[:, C_WSH + goff:C_WSH + goff + gw])
                    nc.gpsimd.collective_compute(
                        "AllGather", OP.bypass,
                        replica_groups=[list(range(NCORES))],
                        ins=[ws_in[:].rearrange("p f -> (p f)")],
                        outs=[ws_gath[:].rearrange("p f -> (p f)")])
                # gather->SBUF loads after ALL collective dispatches, so the
                # later collectives' dispatch isn't blocked behind loads that
                # wait on earlier collectives (Pool queue is in-order)
                for p, (goff, gw) in enumerate(GROUPS):
                    for rr in range(NCORES // 2):
                        src = gaths[p][rr * 256:(rr + 1) * 256, :].rearrange(
                            "(r p) c -> p r c", r=2)
                        nc.gpsimd.dma_start(
                            wsru[:, goff * 8 + rr * 2 * gw:
                                 goff * 8 + (rr + 1) * 2 * gw],
                            src)
            else:
                for q in range(4):
                    nc.sync.dma_start(
                        wsru[:, q * 2048:(q + 1) * 2048],
                        blob_d[:, C_WSH + q * 2048:C_WSH + (q + 1) * 2048])

            h0 = [wp.tile([128, 512], BF16, tag=f"h0{d}", name=f"h0{d}")
                  for d in range(2)]
            h1 = [wp.tile([128, 512], BF16, tag=f"h1{d}", name=f"h1{d}")
                  for d in range(2)]

            # PSUM: 8 banks, all as (128, 512) f32 tiles
            u_ps = {}
            for jj in range(4):
                u_ps[jj] = ps.tile([128, 512], F32, tag=f"u{jj}", name=f"ups{jj}")
            sc_ps = [ps.tile([128, 512], F32, tag=f"sc{h}", name=f"scps{h}")
                     for h in range(2)]
            pn_ps = [ps.tile([128, 512], F32, tag=f"pn{dh}", name=f"pnps{dh}")
                     for dh in range(2)]

            # memr[lp, lh*512+b*256+dh*128+dc] = x[b, lh*128+lp, dh*128+dc]
            #   = PE-transpose of memT block [dp, q] at cols dh*512+lh*256+b*128
            # (emitted in the schedule after the first tanh blocks, so its DVE
            # copies don't delay the first tp adds; needed by pools at ~35us)
            def emit_memr():
                ti = 0
                for lh in range(2):
                    for b in range(2):
                        for dh in range(2):
                            tps = u_ps[ti % 4]
                            src = dh * 512 + lh * 256 + b * 128
                            dst = lh * 512 + b * 256 + dh * 128
                            t32 = bp.tile([128, 128], F32, tag="tsp",
                                          name=f"tsp{ti}")
                            nc.vector.tensor_copy(t32[:],
                                                  memT[:, src:src + 128])
                            nc.tensor.transpose(tps[:, 0:128], t32[:],
                                                ident[:])
                            nc.vector.tensor_copy(memr[:, dst:dst + 128],
                                                  tps[:, 0:128])
                            ti += 1

            # xtT (layer-invariant): contract d per b-half via col-tiling.
            # Staged per 128-l chunk through sc_ps[0] before any scores land.
            xt16 = wp.tile([128, 256], BF16, tag="xt16")
            for ck in range(2):
                co = ck * 256
                for b in range(2):
                    for ci in range(2):
                        nc.tensor.matmul(
                            sc_ps[0][b * 64:(b + 1) * 64, co:co + 128],
                            smalls[:, W1O + (b * 2 + ci) * 64:
                                   W1O + (b * 2 + ci + 1) * 64],
                            memT[:, ci * 512 + co + b * 128:
                                 ci * 512 + co + (b + 1) * 128],
                            start=(ci == 0), stop=(ci == 1))
                nc.vector.tensor_copy(xt16[:, ck * 128:(ck + 1) * 128],
                                      sc_ps[0][:, co:co + 128])

            # per-layer tiles (distinct buffers so layers can overlap)
            yts = [wp.tile([128, 256], F32, tag=f"yt{li}", name=f"yt{li}")
                   for li in range(NL)]
            eTs = [wp.tile([128, 1024], F32, tag=f"eT{li}", name=f"eT{li}")
                   for li in range(NL)]
            rzs = [wp.tile([1, 512], F32, tag=f"rz{li}", name=f"rz{li}")
                   for li in range(NL)]
            rzbs = [wp.tile([128, 512], F32, tag=f"rzb{li}", name=f"rzb{li}")
                    for li in range(NL)]
            poolsTs = [[wp.tile([128, 512], BF16, tag=f"poolsT{li}_{dh}",
                                name=f"poolsT{li}_{dh}") for dh in range(2)]
                       for li in range(NL)]
            gates = {}

            def emit_ytT(li, ck):
                # staged in sc_ps[1][:, ck*256 : ck*256+128]; freed by the yt
                # copy before this layer's chunk-ck score MMs land
                co = ck * 256
                for b in range(2):
                    for ci in range(2):
                        if li == 0:
                            rhs = memT[:, ci * 512 + co + b * 128:
                                       ci * 512 + co + (b + 1) * 128]
                        else:
                            rhs = h0[ci][:, co + b * 128: co + (b + 1) * 128]
                        nc.tensor.matmul(
                            sc_ps[1][b * 64:(b + 1) * 64, co:co + 128],
                            smalls[:, W2O + (b * 2 + ci) * 64:
                                   W2O + (b * 2 + ci + 1) * 64],
                            rhs, start=(ci == 0), stop=(ci == 1))
                nc.vector.tensor_scalar(
                    yts[li][:, ck * 128:(ck + 1) * 128],
                    sc_ps[1][:, co:co + 128], yb[:], None, OP.add)

            def emit_block(li, ck, blk, nj=16):
                co = ck * 256
                tp = bp.tile([128, 256 * nj], BF16, tag="tpre",
                             name=f"tp{li}_{ck}_{blk}", bufs=TPBUFS)
                tb = bp.tile([128, 256 * nj], BF16, tag="tblk",
                             name=f"tb{li}_{ck}_{blk}", bufs=TPBUFS)
                for j in range(nj):
                    s = ck * 128 + blk * nj + j
                    nc.vector.tensor_scalar(
                        tp[:, j * 256:(j + 1) * 256], xt16[:],
                        yts[li][:, s:s + 1], None, OP.add)
                nc.scalar.activation(tb[:], tp[:], AF.Tanh)
                for j in range(nj):
                    q = blk * nj + j
                    for h in range(2):
                        # out cols {co+q, co+128+q}: C-layout b-split
                        nc.tensor.matmul(
                            sc_ps[h][:, co + q: co + q + 129: 128],
                            tb[:, j * 256 + h * 128: j * 256 + (h + 1) * 128],
                            smalls[:, VAO:VAO + 2],
                            start=True, stop=True)

            def emit_pools(li, ck):
                co = ck * 256
                eT, rz, rzb = eTs[li], rzs[li], rzbs[li]
                for h in range(2):
                    nc.scalar.activation(eT[:, h * 512 + co: h * 512 + co + 256],
                                         sc_ps[h][:, co:co + 256], AF.Exp)
                if apply_mask:
                    for h in range(2):
                        for b in range(2):
                            sl = eT[:, h * 512 + co + b * 128:
                                    h * 512 + co + (b + 1) * 128]
                            nc.vector.tensor_scalar(
                                sl, sl, mk[:, h * 2 + b: h * 2 + b + 1],
                                None, OP.mult)
                for h in range(2):
                    nc.tensor.matmul(pn_ps[0][0:1, co:co + 256], onc[:],
                                     eT[:, h * 512 + co: h * 512 + co + 256],
                                     start=(h == 0), stop=(h == 1))
                nc.vector.reciprocal(rz[0:1, co:co + 256],
                                     pn_ps[0][0:1, co:co + 256])
                for b in range(2):
                    nc.tensor.matmul(
                        pn_ps[1][:, co + b * 128: co + (b + 1) * 128], onr[:],
                        rz[0:1, co + b * 128: co + (b + 1) * 128],
                        start=True, stop=True)
                nc.vector.tensor_copy(rzb[:, co:co + 256],
                                      pn_ps[1][:, co:co + 256])
                for dh in range(2):
                    for b in range(2):
                        for lh in range(2):
                            nc.tensor.matmul(
                                pn_ps[dh][:, co + b * 128: co + (b + 1) * 128],
                                memr[:, lh * 512 + b * 256 + dh * 128:
                                     lh * 512 + b * 256 + (dh + 1) * 128],
                                eT[:, lh * 512 + co + b * 128:
                                   lh * 512 + co + (b + 1) * 128],
                                start=(lh == 0), stop=(lh == 1))
                    nc.vector.scalar_tensor_tensor(
                        poolsTs[li][dh][:, co:co + 256],
                        pn_ps[dh][:, co:co + 256],
                        1.0, rzb[:, co:co + 256], OP.mult, OP.mult)

            def emit_sru(li, ck, dr):
                co = ck * 256
                for c in range(4):
                    if c < 2:
                        rhs = (memT[:, c * 512 + co: c * 512 + co + 256]
                               if li == 0 else h0[c][:, co:co + 256])
                    else:
                        rhs = poolsTs[li][c - 2][:, co:co + 256]
                    for jj in range(4):
                        w_off = (((li * 2 + dr) * 16) + c * 4 + jj) * 128
                        nc.tensor.matmul(
                            u_ps[jj][:, co:co + 256],
                            wsru[:, w_off:w_off + 128], rhs,
                            start=(c == 0), stop=(c == 3))
                bcol = (li * 2 + dr) * 2
                if (li, dr) not in gates:
                    gt = {}
                    for nm in ("tf", "f", "g", "bin", "c", "tc2", "tr",
                               "dd", "rd2"):
                        gt[nm] = sp.tile([128, 512], F32, tag=nm,
                                         name=f"{nm}_{li}_{dr}")
                    gates[(li, dr)] = gt
                gt = gates[(li, dr)]
                tf_, f_, g_, bin_, c_, tc2, tr_, dd_, rd2_ = (
                    gt["tf"], gt["f"], gt["g"], gt["bin"], gt["c"],
                    gt["tc2"], gt["tr"], gt["dd"], gt["rd2"])
                nc.scalar.activation(tf_[:, co:co + 256],
                                     u_ps[1][:, co:co + 256], AF.Tanh,
                                     bias=bsru[:, bcol:bcol + 1], scale=0.5)
                nc.vector.tensor_scalar(f_[:, co:co + 256],
                                        tf_[:, co:co + 256], 0.5, 0.5,
                                        OP.mult, OP.add)
                nc.vector.tensor_scalar(g_[:, co:co + 256],
                                        tf_[:, co:co + 256], -0.5, 0.5,
                                        OP.mult, OP.add)
                nc.vector.tensor_tensor(bin_[:, co:co + 256],
                                        g_[:, co:co + 256],
                                        u_ps[0][:, co:co + 256], OP.mult)
                for b in range(2):
                    lo = co + b * 128
                    init = (0.0 if ck == 0
                            else c_[:, lo - 129: lo - 128])
                    nc.vector.tensor_tensor_scan(
                        c_[:, lo:lo + 128], f_[:, lo:lo + 128],
                        bin_[:, lo:lo + 128], init, OP.mult, OP.add)
                nc.scalar.activation(tc2[:, co:co + 256],
                                     c_[:, co:co + 256], AF.Tanh)
                nc.scalar.activation(tr_[:, co:co + 256],
                                     u_ps[2][:, co:co + 256], AF.Tanh,
                                     bias=bsru[:, bcol + 1:bcol + 2],
                                     scale=0.5)
                nc.vector.tensor_tensor(dd_[:, co:co + 256],
                                        tc2[:, co:co + 256],
                                        u_ps[3][:, co:co + 256],
                                        OP.subtract)
                nc.vector.scalar_tensor_tensor(
                    rd2_[:, co:co + 256], tr_[:, co:co + 256], 1.0,
                    dd_[:, co:co + 256], OP.add, OP.mult)
                h_t = h0[dr] if li == 0 else h1[dr]
                nc.vector.scalar_tensor_tensor(
                    h_t[:, co:co + 256], rd2_[:, co:co + 256], 0.5,
                    u_ps[3][:, co:co + 256], OP.mult, OP.add)

            def emit_out(ck, dh):
                co = ck * 256
                nc.sync.dma_start(outT_d[dh, :, co:co + 256],
                                  h1[dh][:, co:co + 256])

            # global schedule across both layers: chunk-1 blocks cover the
            # chunk-0 tails; layer-1 ytT + first blocks are woven into layer
            # 0's tail so ACT never drains at the layer boundary.
            NB = 128 // NJ       # merged blocks per chunk
            emit_ytT(0, 0)
            emit_ytT(0, 1)
            # first block split in two so the first tanh fires ~1.5us earlier
            emit_block(0, 0, 0, NJ // 2)
            emit_block(0, 0, 1, NJ // 2)
            for blk in range(1, NB):
                emit_block(0, 0, blk, NJ)
                if blk == 1:
                    emit_memr()
            emit_block(0, 1, 0, NJ)
            emit_block(0, 1, 1, NJ)
            emit_pools(0, 0)
            for blk in range(2, 5 * NB // 8):
                emit_block(0, 1, blk, NJ)
            emit_sru(0, 0, 0)
            emit_sru(0, 0, 1)
            for blk in range(5 * NB // 8, NB):
                emit_block(0, 1, blk, NJ)
            emit_ytT(1, 0)
            emit_pools(0, 1)
            emit_sru(0, 1, 0)
            emit_sru(0, 1, 1)
            emit_block(1, 0, 0, NJ)
            emit_block(1, 0, 1, NJ)
            emit_ytT(1, 1)
            for blk in range(2, NB):
                emit_block(1, 0, blk, NJ)
            emit_block(1, 1, 0, NJ)
            emit_block(1, 1, 1, NJ)
            emit_pools(1, 0)
            emit_sru(1, 0, 0)
            emit_out(0, 0)
            emit_sru(1, 0, 1)
            emit_out(0, 1)
            for blk in range(2, NB):
                emit_block(1, 1, blk, NJ)
            emit_pools(1, 1)
            emit_sru(1, 1, 0)
            emit_out(1, 0)
            emit_sru(1, 1, 1)
            emit_out(1, 1)

    _split_excess_waits(nc)
    return nc


_CACHE = {}


def _get_nc(apply_mask: bool):
    if apply_mask not in _CACHE:
        _CACHE[apply_mask] = _build(apply_mask)
    return _CACHE[apply_mask]


def _c_layout(arr_pos_b):
    """(pos, b, ...) -> columns in layout C: [ck*256 + b*128 + q]."""
    P2, Bb = arr_pos_b.shape[0], arr_pos_b.shape[1]
    rest = arr_pos_b.shape[2:]
    a = arr_pos_b.reshape(2, 128, Bb, *rest)       # (ck, q, b, ...)
    a = np.moveaxis(a, 2, 1)                       # (ck, b, q, ...)
    return a.reshape(512, *rest)


def make_in_maps(x, x_mask, actions, w1, b1, w2, b2, v,
                 sru_w_f, sru_b_f, sru_w_b, sru_b_b):
    x = np.asarray(x, np.float32)
    x_mask = np.asarray(x_mask)
    actions = np.asarray(actions).astype(np.int64)
    w1 = np.asarray(w1, np.float32); b1 = np.asarray(b1, np.float32)
    w2 = np.asarray(w2, np.float32); b2 = np.asarray(b2, np.float32)
    v = np.asarray(v, np.float32)
    sru_w = [np.asarray(sru_w_f, np.float32), np.asarray(sru_w_b, np.float32)]
    sru_b = [np.asarray(sru_b_f, np.float32), np.asarray(sru_b_b, np.float32)]

    apply_mask = bool(x_mask.any())

    # full wsru weight layout [128, 8192]: 64 blocks of 128 cols keyed
    # (li, dr, c, jj); sharded per core as cols [r*1024:(r+1)*1024]
    wsru_full = np.empty((128, 8192), np.float32)
    for li in range(NL):
        for dr in range(2):
            blk = sru_w[dr][li].reshape(4, 128, 4, 128)  # [c, dp, jj, m]
            for c in range(4):
                for jj in range(4):
                    off = ((li * 2 + dr) * 16 + c * 4 + jj) * 128
                    wsru_full[:, off:off + 128] = blk[c, :, jj, :]
    wsru16 = wsru_full.astype(BF16_NP)
    bsru = np.empty((128, 8), np.float32)
    for li in range(NL):
        for dr in range(2):
            bb = sru_b[dr][li]
            bsru[:, (li * 2 + dr) * 2 + 0] = 0.5 * bb[0:128]
            bsru[:, (li * 2 + dr) * 2 + 1] = 0.5 * bb[128:256]
    bsru16 = bsru.astype(BF16_NP)

    in_maps = []
    for core in range(NCORES):
        gb = [B2 * core + b for b in range(B2)]
        xs = x[gb]  # (2, S, D)
        a = [int(actions[g]) for g in gb]
        blob = np.zeros((128, C_END), np.float32)
        # memT[dp, dh*512 + C(l, b)] = x[b, l, dh*128+dp]
        arr = xs.transpose(1, 0, 2)                # (l, b, d)
        colsC = _c_layout(arr)                     # (512C, d)
        for dh in range(2):
            blob[:, C_MEMT + dh * 512:C_MEMT + (dh + 1) * 512] = \
                colsC[:, dh * 128:(dh + 1) * 128].T
        # w dense: [128 dp(ci-half), (b,ci) block * 64 + k]
        for b in range(2):
            for ci in range(2):
                cc = b * 2 + ci
                blob[:, C_W1 + cc * 64:C_W1 + (cc + 1) * 64] = \
                    w1[a[b], ci * 128:(ci + 1) * 128, :]
                blob[:, C_W2 + cc * 64:C_W2 + (cc + 1) * 64] = \
                    w2[a[b], ci * 128:(ci + 1) * 128, :]
        for b in range(2):
            blob[b * 64:(b + 1) * 64, C_VA + b] = v[a[b]]
            blob[b * 64:(b + 1) * 64, C_YB] = b1[a[b]] + b2[a[b]]
        blob[:, C_BS:C_BS + 8] = bsru
        if apply_mask:
            for lh in range(2):
                for b in range(2):
                    blob[:, C_MK + lh * 2 + b] = np.where(
                        x_mask[gb[b], lh * 128:(lh + 1) * 128], 0.0, 1.0)
        blob16 = blob.astype(BF16_NP)
        if USE_ALLGATHER:
            # shard-col goff..goff+gw maps to wsru block cols goff*8..
            for goff, gw in GROUPS:
                blob16[:, C_WSH + goff:C_WSH + goff + gw] = \
                    wsru16[:, goff * 8 + core * gw:goff * 8 + (core + 1) * gw]
        else:
            blob16[:, C_WSH:C_WSH + 8192] = wsru16
        blob16[:, C_BS:C_BS + 8] = bsru16
        in_maps.append({"blob": blob16})
    return in_maps, apply_mask


def assemble_output(results):
    y = np.empty((B, S, D), np.float32)
    for core in range(NCORES):
        outT = np.asarray(results[core]["outT"]).astype(np.float32)
        oc = outT.reshape(2, 128, 2, 2, 128)       # [dh, dp, ck, b, q]
        for b in range(B2):
            # y[b, s, dh*128+dp]; s = ck*128+q
            yb = oc[:, :, :, b, :]                 # (dh, dp, ck, q)
            yb = yb.transpose(2, 3, 0, 1).reshape(S, D)
            y[B2 * core + b] = yb
    return y


def kernel(**inputs) -> np.ndarray:
    in_maps, apply_mask = make_in_maps(**inputs)
    nc = _get_nc(apply_mask)
    res = run_bass_kernel_spmd(nc, in_maps, list(range(NCORES)))
    return assemble_output(res.results)
